# revision 10
# baseline (speedup 1.0000x reference)
"""Cross-attention kernel for Trainium2, data-parallel over batch on 8 cores.

Per core (one batch element):
  Q = x @ Wq + bq ; K = e @ Wk + bk ; V = e @ Wv + bv
  out = softmax(Q K^T / 8) @ V

Fast path (biases all zero, which is what setup_inputs produces) uses the
association S = x (Wq Wk^T) e^T:
  G  = Wq @ Wk^T          [d_in, d_in]   one 1024^3 GEMM, replaces K-proj
  HT = G^T @ xT           [d_in_e, sq]
  ST = eT^T @ HT          [skv, sq]      fused with V = e @ Wv (shared eT
                                         stationary operand)
  PT = exp(ST/8)          bf16           (no max subtraction: |s/8| < ~25)
  out = (PT^T @ V) * 1/(PT^T @ 1)

All input transposes (x, e, Wq, Wk) run on the DMA XBAR engine
(dma_start_transpose, 2-byte dtype) instead of the PE, so the tensor engine
only executes the five GEMM phases. Inputs are loaded fp32, cast to fp16 on
DVE, then XBAR-transposed into flat [128, 8*S] layouts:
  xta[p, d, s] = x[s, d*128+p]   (view [128, 8, 2048])
  eta          likewise for e
  wqta[p, a, c] = wq[c, a*128+p] (view [128, 8, 1024]), wkta likewise

Numerics: fp16 projections + bf16 probs/V gives ~2e-3 rel L2 error vs the
fp32 reference (bf16 everywhere would be ~1e-2).
"""

import numpy as np

import concourse.bacc as bacc
import concourse.bass as bass
import concourse.mybir as mybir
import concourse.tile as tile
from concourse.bass_utils import run_bass_kernel_spmd
from concourse.masks import make_identity

P = 128
D = 1024
ND = D // P  # 8 d tiles
SQ = 2048
NSQ = SQ // P  # 16
SKV = 2048
NSKV = SKV // P  # 16
NC = SQ // 512  # 4 strips of 512 along s
N_CORES = 8

F32 = mybir.dt.float32
F16 = mybir.dt.float16
BF16 = mybir.dt.bfloat16
AF = mybir.ActivationFunctionType


def build(reps=1, fast=False):
    nc = bacc.Bacc("TRN2", target_bir_lowering=False, debug=False)
    x = nc.declare_dram_parameter("x", [SQ, D], F32, isOutput=False)
    e = nc.declare_dram_parameter("e", [SKV, D], F32, isOutput=False)
    wq = nc.declare_dram_parameter("wq", [D, D], F32, isOutput=False)
    wk = nc.declare_dram_parameter("wk", [D, D], F32, isOutput=False)
    wv = nc.declare_dram_parameter("wv", [D, D], F32, isOutput=False)
    bq = nc.declare_dram_parameter("bq", [D], F32, isOutput=False)
    bk = nc.declare_dram_parameter("bk", [D], F32, isOutput=False)
    bv = nc.declare_dram_parameter("bv", [D], F32, isOutput=False)
    out = nc.declare_dram_parameter("out", [SQ, D], F32, isOutput=True)

    with tile.TileContext(nc) as tc:
        for _rep in range(reps):
            if fast:
                _emit_body_fast(nc, tc, x, e, wq, wk, wv, out)
            else:
                _emit_body(nc, tc, x, e, wq, wk, wv, bq, bk, bv, out)

    nc.compile()
    return nc


def _emit_body_fast(nc, tc, x, e, wq, wk, wv, out):
    # ---- left-stack pools (released LIFO) ----
    const = tc.alloc_tile_pool(name="const", bufs=1, side="left")
    eta_p = tc.alloc_tile_pool(name="eta", bufs=1, side="left")
    hta_p = tc.alloc_tile_pool(name="hta", bufs=1, side="left")
    wva_p = tc.alloc_tile_pool(name="wva", bufs=1, side="left")
    xta_p = tc.alloc_tile_pool(name="xta", bufs=1, side="left")
    ga_p = tc.alloc_tile_pool(name="ga", bufs=1, side="left")
    wqta_p = tc.alloc_tile_pool(name="wqta", bufs=1, side="left")
    wkta_p = tc.alloc_tile_pool(name="wkta", bufs=1, side="left")
    ld_p = tc.alloc_tile_pool(name="ld", bufs=6, side="left")
    c16_p = tc.alloc_tile_pool(name="c16", bufs=6, side="left")

    ones_col = const.tile([P, 1], BF16, tag="ones_col")
    nc.gpsimd.memset(ones_col[:], 1.0)

    # flat transposed layouts; [p, t, s] views
    eta = eta_p.tile([P, ND * SKV], F16, tag="eta")
    eta_v = eta[:].rearrange("p (a s) -> p a s", a=ND)
    hta = hta_p.tile([P, ND * SQ], F16, tag="hta")
    hta_v = hta[:].rearrange("p (a s) -> p a s", a=ND)
    wva = wva_p.tile([P, ND * D], F16, tag="wva")
    wva_v = wva[:].rearrange("p (a s) -> p a s", a=ND)
    xta = xta_p.tile([P, ND * SQ], F16, tag="xta")
    xta_v = xta[:].rearrange("p (a s) -> p a s", a=ND)
    ga = ga_p.tile([P, ND * D], F16, tag="ga")
    ga_v = ga[:].rearrange("p (a s) -> p a s", a=ND)
    wqta = wqta_p.tile([P, ND * D], F16, tag="wqta")
    wqta_v = wqta[:].rearrange("p (a s) -> p a s", a=ND)
    wkta = wkta_p.tile([P, ND * D], F16, tag="wkta")
    wkta_v = wkta[:].rearrange("p (a s) -> p a s", a=ND)

    # ---- input chains ----
    # The tile framework assigns DMA ops to the 8 HWDGE lanes round-robin in
    # emission order; a DMA waits for completion of the DMA 8 slots earlier
    # (lane recycle). So emit DMAs in kind-homogeneous batches of 8,
    # software-pipelined: a tensor's loads go ~2 batches before its
    # transposes so the casts are done by the time the transpose batch's
    # lane-waits clear.
    def load_batch(src_dram, tag, js):
        tiles = {}
        for j in js:
            lt = ld_p.tile([P, D], F32, name=f"ld_{tag}{j}", tag="ld")
            nc.sync.dma_start(lt[:], src_dram.ap()[j * P : (j + 1) * P, :])
            tiles[j] = lt
        return tiles

    def cast_batch(lds, tag, js, dst_v=None):
        """DVE cast fp32->fp16; into c16 staging (for transposes) or straight
        into dst_v[:, j, :] (untransposed weights)."""
        cts = {}
        for j in js:
            if dst_v is not None:
                nc.vector.tensor_copy(dst_v[:, j, :], lds[j][:])
            else:
                ct = c16_p.tile([P, D], F16, name=f"c16_{tag}{j}", tag="c16")
                nc.vector.tensor_copy(ct[:], lds[j][:])
                cts[j] = ct
        return cts

    def tr_batch(cts, dst_v, js):
        for j in js:
            nc.sync.dma_start_transpose(dst_v[:, :, j * P : (j + 1) * P], cts[j][:])

    J8 = list(range(8))
    wq_ld = load_batch(wq, "wq", J8)
    wq_ct = cast_batch(wq_ld, "wq", J8)
    wk_ld = load_batch(wk, "wk", J8)
    wk_ct = cast_batch(wk_ld, "wk", J8)
    tr_batch(wq_ct, wqta_v, J8)
    tr_batch(wk_ct, wkta_v, J8)
    x0_ld = load_batch(x, "x", list(range(0, 8)))
    x0_ct = cast_batch(x0_ld, "x", list(range(0, 8)))
    tr_batch(x0_ct, xta_v, list(range(0, 8)))

    # ---- G = Wq @ Wk^T ----
    ps_g = tc.alloc_tile_pool(name="ps_g", bufs=2, space="PSUM")
    for m in range(ND):
        psg = ps_g.tile([P, D], F32, name="psg", tag="psg")
        for k in range(ND):
            lhsT = wqta_v[:, k, m * P : (m + 1) * P]
            for h in range(2):
                nc.tensor.matmul(
                    psg[:, h * 512 : (h + 1) * 512],
                    lhsT,
                    wkta_v[:, k, h * 512 : (h + 1) * 512],
                    start=(k == 0),
                    stop=(k == ND - 1),
                )
        nc.vector.tensor_copy(ga_v[:, m, :], psg[:])
    ps_g.release()

    # ---- HT = G^T @ xT, per 512-wide sq chunk ----
    # remaining pipelined chains: x tail, e, wv. ST's kt loop only needs
    # e-transpose j for kt=j, so the e1 batch never gates the ST start.
    x1_ld = load_batch(x, "x", list(range(8, 16)))
    x1_ct = cast_batch(x1_ld, "x", list(range(8, 16)))
    tr_batch(x1_ct, xta_v, list(range(8, 16)))
    e0_ld = load_batch(e, "e", list(range(0, 8)))
    e0_ct = cast_batch(e0_ld, "e", list(range(0, 8)))
    tr_batch(e0_ct, eta_v, list(range(0, 8)))
    wv_ld = load_batch(wv, "wv", J8)
    cast_batch(wv_ld, "wv", J8, dst_v=wva_v)
    e1_ld = load_batch(e, "e", list(range(8, 16)))
    e1_ct = cast_batch(e1_ld, "e", list(range(8, 16)))
    tr_batch(e1_ct, eta_v, list(range(8, 16)))

    ps_proj = tc.alloc_tile_pool(name="ps_proj", bufs=2, space="PSUM")
    for c in range(NC):
        for dot2 in range(0, ND, 2):
            psq = ps_proj.tile([P, D], F32, name="psq", tag="psq")
            for dit in range(ND):
                rhs = xta_v[:, dit, c * 512 : (c + 1) * 512]
                for u in range(2):
                    nc.tensor.matmul(
                        psq[:, u * 512 : (u + 1) * 512],
                        ga_v[:, dit, (dot2 + u) * P : (dot2 + u + 1) * P],
                        rhs,
                        start=(dit == 0),
                        stop=(dit == ND - 1),
                    )
            nc.scalar.activation(
                hta_v[:, dot2 : dot2 + 2, c * 512 : (c + 1) * 512],
                psq[:].rearrange("p (a s) -> p a s", a=2),
                AF.Identity,
            )
    ps_proj.release()

    # left stack teardown (LIFO): everything above eta/hta/wva is dead once
    # the e-chain and HT have drained; releasing here lets pt/v reuse it.
    c16_p.release()
    ld_p.release()
    wkta_p.release()
    wqta_p.release()
    ga_p.release()
    xta_p.release()

    # ---- attention: fused ST + V (shared eT stationaries), exp, then PV ----
    pt_pool = tc.alloc_tile_pool(name="pt", bufs=NSKV, side="right")
    v_pool = tc.alloc_tile_pool(name="v", bufs=NSKV, side="right")

    pT = [pt_pool.tile([P, SQ], BF16, name=f"pT{t}", tag="pT") for t in range(NSKV)]
    vt = [v_pool.tile([P, D], BF16, name=f"v{t}", tag="v") for t in range(NSKV)]
    ps_st = tc.alloc_tile_pool(name="ps_st", bufs=2, space="PSUM")
    ps_v = tc.alloc_tile_pool(name="ps_v", bufs=2, space="PSUM")
    for kt_i in range(NSKV):
        pa = ps_st.tile([P, 1024], F32, name="pss_a", tag="pss_st")
        pb = ps_st.tile([P, 1024], F32, name="pss_b", tag="pss_st")
        psv = ps_v.tile([P, D], F32, name="psv", tag="psv")
        st_tiles = (pa, pa, pb, pb)
        for dit in range(ND):
            lhsT = eta_v[:, dit, kt_i * P : (kt_i + 1) * P]
            first = dit == 0
            last = dit == ND - 1
            for c in range(NC):
                nc.tensor.matmul(
                    st_tiles[c][:, (c % 2) * 512 : (c % 2 + 1) * 512],
                    lhsT,
                    hta_v[:, dit, c * 512 : (c + 1) * 512],
                    start=first,
                    stop=last,
                )
            for h in range(2):
                nc.tensor.matmul(
                    psv[:, h * 512 : (h + 1) * 512],
                    lhsT,
                    wva_v[:, dit, h * 512 : (h + 1) * 512],
                    start=first,
                    stop=last,
                )
        for c in range(NC):
            nc.scalar.activation(
                pT[kt_i][:, c * 512 : (c + 1) * 512],
                st_tiles[c][:, (c % 2) * 512 : (c % 2 + 1) * 512],
                AF.Exp,
                scale=0.125,
            )
        for h in range(2):
            nc.vector.tensor_copy(
                vt[kt_i][:, h * 512 : (h + 1) * 512],
                psv[:, h * 512 : (h + 1) * 512],
            )
    ps_v.release()
    ps_st.release()

    # ---- PV: out = (PT^T @ V) / (PT^T @ 1) ----
    outp = tc.alloc_tile_pool(name="outp", bufs=3, side="right")
    small = tc.alloc_tile_pool(name="small", bufs=4, side="right")
    ps_pv = tc.alloc_tile_pool(name="ps_pv", bufs=2, space="PSUM")
    ps_sum = tc.alloc_tile_pool(name="ps_sum", bufs=1, space="PSUM")
    for sqt in range(NSQ):
        pso = ps_pv.tile([P, D], F32, name="pso", tag="pso")
        psum_s = ps_sum.tile([P, 1], F32, name="psum_s", tag="psum_s")
        for kt_i in range(NSKV):
            lhsT = pT[kt_i][:, sqt * P : (sqt + 1) * P]
            first = kt_i == 0
            last = kt_i == NSKV - 1
            for hh in range(2):
                nc.tensor.matmul(
                    pso[:, hh * 512 : (hh + 1) * 512],
                    lhsT,
                    vt[kt_i][:, hh * 512 : (hh + 1) * 512],
                    start=first,
                    stop=last,
                )
            nc.tensor.matmul(psum_s[:], lhsT, ones_col[:], start=first, stop=last)
        recip = small.tile([P, 1], F32, name="recip", tag="recip")
        nc.vector.reciprocal(recip[:], psum_s[:])
        ot = outp.tile([P, D], F32, name="ot", tag="ot")
        nc.vector.tensor_scalar_mul(ot[:], pso[:], recip[:])
        nc.sync.dma_start(out.ap()[sqt * P : (sqt + 1) * P, :], ot[:])

    ps_sum.release()
    ps_pv.release()
    small.release()
    outp.release()
    v_pool.release()
    pt_pool.release()
    wva_p.release()
    hta_p.release()
    eta_p.release()
    const.release()


def _emit_body(nc, tc, x, e, wq, wk, wv, bq, bk, bv, out):
    """General path (nonzero biases). Unchanged from the baseline kernel."""
    # ---- left-stack pools (released LIFO) ----
    const = tc.alloc_tile_pool(name="const", bufs=1, side="left")
    qt_pool = tc.alloc_tile_pool(name="qt", bufs=ND, side="left")
    kt_pool = tc.alloc_tile_pool(name="kt", bufs=ND, side="left")
    w16_pool = tc.alloc_tile_pool(name="w16", bufs=16, side="left")
    et_pool = tc.alloc_tile_pool(name="et", bufs=ND, side="left")
    ldW = tc.alloc_tile_pool(name="ldW", bufs=3, side="left")
    ldE = tc.alloc_tile_pool(name="ldE", bufs=4, side="left")
    xl16_pool = tc.alloc_tile_pool(name="xl16", bufs=4, side="left")
    ps_proj = tc.alloc_tile_pool(name="ps_proj", bufs=4, space="PSUM")
    ps_tr = tc.alloc_tile_pool(name="ps_tr", bufs=4, space="PSUM")

    def _load_w16(w_dram):
        tiles = []
        for dit in range(ND):
            wl = ldW.tile([P, D], F32, name=f"wl_{w_dram.name}_{dit}", tag="ldw")
            nc.sync.dma_start(wl[:], w_dram.ap()[dit * P : (dit + 1) * P, :])
            w16t = w16_pool.tile([P, D], F16, name=f"w16_{w_dram.name}_{dit}", tag="w16")
            nc.vector.tensor_copy(w16t[:], wl[:])
            tiles.append(w16t)
        return tiles

    identity = const.tile([P, P], F16, tag="ident")
    make_identity(nc, identity[:])
    ones_row = const.tile([1, P], F16, tag="ones_row")
    nc.gpsimd.memset(ones_row[:], 1.0)
    ones_col = const.tile([P, 1], BF16, tag="ones_col")
    nc.gpsimd.memset(ones_col[:], 1.0)
    bqt = const.tile([P, ND], F32, tag="bqt")
    nc.sync.dma_start(bqt[:], bq.ap().rearrange("(t p) -> p t", p=P))
    bkt = const.tile([P, ND], F32, tag="bkt")
    nc.sync.dma_start(bkt[:], bk.ap().rearrange("(t p) -> p t", p=P))
    bvl = ldW.tile([1, D], F32, tag="ldw")
    nc.sync.dma_start(bvl[:], bv.ap().rearrange("(a n) -> a n", a=1))
    bv16 = const.tile([1, D], F16, tag="bv16")
    nc.vector.tensor_copy(bv16[:], bvl[:])

    def transpose_group(ld_tiles, dst_write, tag):
        l16 = []
        for j in range(4):
            t16 = xl16_pool.tile([P, D], F16, name=f"l16_{tag}_{j}", tag="l16")
            nc.vector.tensor_copy(t16[:], ld_tiles[j][:])
            l16.append(t16)
        for dit2 in range(0, ND, 2):
            psts = [
                ps_tr.tile([P, 512], F16, name=f"pst_{tag}{u}", tag="pst")
                for u in range(2)
            ]
            for j in range(4):
                for u in range(2):
                    nc.tensor.matmul(
                        psts[u][:, j * P : (j + 1) * P],
                        l16[j][:, (dit2 + u) * P : (dit2 + u + 1) * P],
                        identity[:],
                        is_transpose=True,
                        start=(j == 0),
                        stop=(j == 3),
                    )
            for u in range(2):
                dst_write(dit2 + u, psts[u])

    def project_chunk(w16, rhs_of_dit, dst_tiles, bias_cols, c):
        for dot2 in range(0, ND, 2):
            psq = [
                ps_proj.tile([P, 512], F32, name=f"psq{u}", tag="psp")
                for u in range(2)
            ]
            for dit in range(ND):
                for u in range(2):
                    nc.tensor.matmul(
                        psq[u][:],
                        w16[dit][:, (dot2 + u) * P : (dot2 + u + 1) * P],
                        rhs_of_dit(dit),
                        start=(dit == 0),
                        stop=(dit == ND - 1),
                    )
            for u in range(2):
                nc.scalar.activation(
                    dst_tiles[dot2 + u][:, c * 512 : (c + 1) * 512],
                    psq[u][:],
                    AF.Identity,
                    bias=bias_cols[:, dot2 + u : dot2 + u + 1],
                )

    # ---- x -> xT chunks -> QT, interleaved per 512-chunk ----
    xtc_pool = tc.alloc_tile_pool(name="xtc", bufs=2 * ND, side="left")
    ldX = tc.alloc_tile_pool(name="ldX", bufs=4, side="left")

    def load_group(pool, src_dram, c, tag):
        tiles = []
        for j in range(4):
            st = c * 4 + j
            t = pool.tile([P, D], F32, name=f"{tag}{c}_{j}", tag=tag)
            nc.sync.dma_start(t[:], src_dram.ap()[st * P : (st + 1) * P, :])
            tiles.append(t)
        return tiles

    qT = [qt_pool.tile([P, SQ], F16, name=f"qT{d}", tag="qT") for d in range(ND)]
    xg = {0: load_group(ldX, x, 0, "ldx")}
    wq16 = _load_w16(wq)
    eg = {0: load_group(ldE, e, 0, "lde")}
    for c in range(NC):
        if c + 1 < NC:
            xg[c + 1] = load_group(ldX, x, c + 1, "ldx")
        xtc = [
            xtc_pool.tile([P, 512], F16, name=f"xtc{c}_{d}", tag="xtc")
            for d in range(ND)
        ]

        def wr_x(dit, pst, xtc=xtc):
            nc.vector.tensor_copy(xtc[dit][:], pst[:])

        transpose_group(xg.pop(c), wr_x, "x")
        project_chunk(wq16, lambda dit, xtc=xtc: xtc[dit][:], qT, bqt, c)
    ldX.release()
    xtc_pool.release()

    # ---- e -> eT (kept resident) -> KT, interleaved per 512-chunk ----
    eT = [et_pool.tile([P, SKV], F16, name=f"eT{d}", tag="eT") for d in range(ND)]
    kT = [kt_pool.tile([P, SKV], F16, name=f"kT{d}", tag="kT") for d in range(ND)]
    wk16 = _load_w16(wk)
    for c in range(NC):
        if c + 1 < NC:
            eg[c + 1] = load_group(ldE, e, c + 1, "lde")

        def wr_e(dit, pst, c=c):
            nc.vector.tensor_copy(eT[dit][:, c * 512 : (c + 1) * 512], pst[:])

        transpose_group(eg.pop(c), wr_e, "e")
        project_chunk(
            wk16,
            lambda dit, c=c: eT[dit][:, c * 512 : (c + 1) * 512],
            kT,
            bkt,
            c,
        )
    xl16_pool.release()
    ldE.release()

    # ---- Wv ; V ----
    wv16 = _load_w16(wv)
    ldW.release()
    ps_tr.release()

    v_pool = tc.alloc_tile_pool(name="v", bufs=NSKV, side="right")
    vt = [v_pool.tile([P, D], BF16, name=f"v{t}", tag="v") for t in range(NSKV)]
    for kt_i in range(NSKV):
        ps_half = []
        for h in range(2):
            psv = ps_proj.tile([P, 512], F32, name=f"psv{h}", tag="psp")
            nc.tensor.matmul(
                psv[:],
                ones_row[:],
                bv16[:, h * 512 : (h + 1) * 512],
                start=True,
                stop=False,
            )
            ps_half.append(psv)
        for dit in range(ND):
            for h in range(2):
                nc.tensor.matmul(
                    ps_half[h][:],
                    eT[dit][:, kt_i * P : (kt_i + 1) * P],
                    wv16[dit][:, h * 512 : (h + 1) * 512],
                    start=False,
                    stop=(dit == ND - 1),
                )
        for h in range(2):
            nc.vector.tensor_copy(vt[kt_i][:, h * 512 : (h + 1) * 512], ps_half[h][:])

    ps_proj.release()
    et_pool.release()
    w16_pool.release()

    # ---- attention: ST+exp phase (full PT materialized), then PV phase ----
    pt_pool = tc.alloc_tile_pool(name="pt", bufs=NSKV, side="right")
    outp = tc.alloc_tile_pool(name="outp", bufs=3, side="right")
    small = tc.alloc_tile_pool(name="small", bufs=4, side="right")

    pT = [pt_pool.tile([P, SQ], BF16, name=f"pT{t}", tag="pT") for t in range(NSKV)]
    ps_st = tc.alloc_tile_pool(name="ps_st", bufs=2, space="PSUM")
    for kt_i in range(NSKV):
        pss = ps_st.tile([P, SQ], F32, name="pss_st", tag="pss_st")
        for dit in range(ND):
            lhsT = kT[dit][:, kt_i * P : (kt_i + 1) * P]
            for c in range(NC):
                nc.tensor.matmul(
                    pss[:, c * 512 : (c + 1) * 512],
                    lhsT,
                    qT[dit][:, c * 512 : (c + 1) * 512],
                    start=(dit == 0),
                    stop=(dit == ND - 1),
                )
        for c in range(NC):
            nc.scalar.activation(
                pT[kt_i][:, c * 512 : (c + 1) * 512],
                pss[:, c * 512 : (c + 1) * 512],
                AF.Exp,
                scale=0.125,
            )
    ps_st.release()

    ps_pv = tc.alloc_tile_pool(name="ps_pv", bufs=2, space="PSUM")
    ps_sum = tc.alloc_tile_pool(name="ps_sum", bufs=2, space="PSUM")
    for sqt in range(NSQ):
        pso = ps_pv.tile([P, D], F32, name="pso", tag="pso")
        psum_s = ps_sum.tile([P, 1], F32, name="psum_s", tag="psum_s")
        for kt_i in range(NSKV):
            lhsT = pT[kt_i][:, sqt * P : (sqt + 1) * P]
            first = kt_i == 0
            last = kt_i == NSKV - 1
            for h in range(2):
                nc.tensor.matmul(
                    pso[:, h * 512 : (h + 1) * 512],
                    lhsT,
                    vt[kt_i][:, h * 512 : (h + 1) * 512],
                    start=first,
                    stop=last,
                )
            nc.tensor.matmul(psum_s[:], lhsT, ones_col[:], start=first, stop=last)
        recip = small.tile([P, 1], F32, name="recip", tag="recip")
        nc.vector.reciprocal(recip[:], psum_s[:])
        ot = outp.tile([P, D], F32, name="ot", tag="ot")
        nc.vector.tensor_scalar_mul(ot[:], pso[:], recip[:])
        nc.sync.dma_start(out.ap()[sqt * P : (sqt + 1) * P, :], ot[:])

    ps_sum.release()
    ps_pv.release()
    small.release()
    outp.release()
    pt_pool.release()
    v_pool.release()
    kt_pool.release()
    qt_pool.release()
    const.release()


_NC_CACHE = {}


def _get_nc(fast):
    if fast not in _NC_CACHE:
        _NC_CACHE[fast] = build(fast=fast)
    return _NC_CACHE[fast]


def kernel(
    hidden_states,
    encoder_hidden_states,
    Wq,
    bq,
    Wk,
    bk,
    Wv,
    bv,
    _trace=False,
    _trace_kwargs=None,
):
    hs = np.ascontiguousarray(np.asarray(hidden_states, np.float32))
    es = np.ascontiguousarray(np.asarray(encoder_hidden_states, np.float32))
    wq_ = np.ascontiguousarray(np.asarray(Wq, np.float32))
    wk_ = np.ascontiguousarray(np.asarray(Wk, np.float32))
    wv_ = np.ascontiguousarray(np.asarray(Wv, np.float32))
    bq_ = np.ascontiguousarray(np.asarray(bq, np.float32))
    bk_ = np.ascontiguousarray(np.asarray(bk, np.float32))
    bv_ = np.ascontiguousarray(np.asarray(bv, np.float32))

    # The S = x (Wq Wk^T) e^T association only absorbs the biases when they
    # are zero; fall back to the general module otherwise.
    fast = not (bq_.any() or bk_.any() or bv_.any())
    nc = _get_nc(fast)
    in_maps = [
        {
            "x": hs[c],
            "e": es[c],
            "wq": wq_,
            "wk": wk_,
            "wv": wv_,
            "bq": bq_,
            "bk": bk_,
            "bv": bv_,
        }
        for c in range(N_CORES)
    ]
    res = run_bass_kernel_spmd(
        nc,
        in_maps,
        list(range(N_CORES)),
        trace=_trace,
        **(_trace_kwargs or {}),
    )
    out = np.stack([res.results[c]["out"] for c in range(N_CORES)], axis=0)
    if _trace:
        return out, res
    return out


# revision 19
# speedup vs baseline: 1.0675x; 1.0675x over previous
"""Cross-attention kernel for Trainium2, data-parallel over batch on 8 cores.

Per core (one batch element):
  Q = x @ Wq + bq ; K = e @ Wk + bk ; V = e @ Wv + bv
  out = softmax(Q K^T / 8) @ V

Fast path (biases all zero, which is what setup_inputs produces) uses the
association S = x (Wq Wk^T) e^T:
  G  = Wq @ Wk^T          [d_in, d_in]   one 1024^3 GEMM, replaces K-proj
  HT = G^T @ xT           [d_in_e, sq]
  ST = eT^T @ HT          [skv, sq]      fused with V = e @ Wv (shared eT
                                         stationary operand)
  PT = exp(ST/8)          bf16           (no max subtraction: |s/8| < ~25)
  out = (PT^T @ V) * 1/(PT^T @ 1)

All input transposes (x, e, Wq, Wk) run on the DMA XBAR engine
(dma_start_transpose, 2-byte dtype) instead of the PE, so the tensor engine
only executes the five GEMM phases. Inputs are loaded fp32, cast to fp16 on
DVE, then XBAR-transposed into flat [128, 8*S] layouts:
  xta[p, d, s] = x[s, d*128+p]   (view [128, 8, 2048])
  eta          likewise for e
  wqta[p, a, c] = wq[c, a*128+p] (view [128, 8, 1024]), wkta likewise

Numerics: fp16 projections + bf16 probs/V gives ~2e-3 rel L2 error vs the
fp32 reference (bf16 everywhere would be ~1e-2).
"""

import numpy as np

import concourse.bacc as bacc
import concourse.bass as bass
import concourse.mybir as mybir
import concourse.tile as tile
from concourse.bass_utils import run_bass_kernel_spmd
from concourse.masks import make_identity

P = 128
D = 1024
ND = D // P  # 8 d tiles
SQ = 2048
NSQ = SQ // P  # 16
SKV = 2048
NSKV = SKV // P  # 16
NC = SQ // 512  # 4 strips of 512 along s
N_CORES = 8

F32 = mybir.dt.float32
F16 = mybir.dt.float16
BF16 = mybir.dt.bfloat16
AF = mybir.ActivationFunctionType


def build(reps=1, fast=False):
    nc = bacc.Bacc("TRN2", target_bir_lowering=False, debug=False)
    x = nc.declare_dram_parameter("x", [SQ, D], F32, isOutput=False)
    e = nc.declare_dram_parameter("e", [SKV, D], F32, isOutput=False)
    wq = nc.declare_dram_parameter("wq", [D, D], F32, isOutput=False)
    wk = nc.declare_dram_parameter("wk", [D, D], F32, isOutput=False)
    wv = nc.declare_dram_parameter("wv", [D, D], F32, isOutput=False)
    bq = nc.declare_dram_parameter("bq", [D], F32, isOutput=False)
    bk = nc.declare_dram_parameter("bk", [D], F32, isOutput=False)
    bv = nc.declare_dram_parameter("bv", [D], F32, isOutput=False)
    out = nc.declare_dram_parameter("out", [SQ, D], F32, isOutput=True)

    with tile.TileContext(nc) as tc:
        for _rep in range(reps):
            if fast:
                _emit_body_fast(nc, tc, x, e, wq, wk, wv, out)
            else:
                _emit_body(nc, tc, x, e, wq, wk, wv, bq, bk, bv, out)

    nc.compile()
    return nc


def _emit_body_fast(nc, tc, x, e, wq, wk, wv, out):
    # ---- left-stack pools (released LIFO) ----
    const = tc.alloc_tile_pool(name="const", bufs=1, side="left")
    eta_p = tc.alloc_tile_pool(name="eta", bufs=1, side="left")
    hta_p = tc.alloc_tile_pool(name="hta", bufs=1, side="left")
    wva_p = tc.alloc_tile_pool(name="wva", bufs=1, side="left")
    xta_p = tc.alloc_tile_pool(name="xta", bufs=1, side="left")
    ga_p = tc.alloc_tile_pool(name="ga", bufs=1, side="left")
    wqta_p = tc.alloc_tile_pool(name="wqta", bufs=1, side="left")
    wkta_p = tc.alloc_tile_pool(name="wkta", bufs=1, side="left")
    w16_p = tc.alloc_tile_pool(name="w16", bufs=ND, side="left")
    ld_p = tc.alloc_tile_pool(name="ld", bufs=4, side="left")
    c16_p = tc.alloc_tile_pool(name="c16", bufs=5, side="left")

    ones_col = const.tile([P, 1], BF16, tag="ones_col")
    nc.gpsimd.memset(ones_col[:], 1.0)
    identity = const.tile([P, P], F16, tag="ident")
    make_identity(nc, identity[:])

    # flat transposed layouts; [p, t, s] views
    eta = eta_p.tile([P, ND * SKV], F16, tag="eta")
    eta_v = eta[:].rearrange("p (a s) -> p a s", a=ND)
    hta = hta_p.tile([P, ND * SQ], F16, tag="hta")
    hta_v = hta[:].rearrange("p (a s) -> p a s", a=ND)
    wva = wva_p.tile([P, ND * D], F16, tag="wva")
    wva_v = wva[:].rearrange("p (a s) -> p a s", a=ND)
    xta = xta_p.tile([P, ND * SQ], F16, tag="xta")
    xta_v = xta[:].rearrange("p (a s) -> p a s", a=ND)
    ga = ga_p.tile([P, ND * D], F16, tag="ga")
    ga_v = ga[:].rearrange("p (a s) -> p a s", a=ND)
    wqta = wqta_p.tile([P, ND * D], F16, tag="wqta")
    wqta_v = wqta[:].rearrange("p (a s) -> p a s", a=ND)
    wkta = wkta_p.tile([P, ND * D], F16, tag="wkta")
    wkta_v = wkta[:].rearrange("p (a s) -> p a s", a=ND)

    # ---- input chains ----
    # The tile framework assigns DMA ops to the 8 HWDGE lanes round-robin in
    # emission order; a DMA waits for completion of the DMA 8 slots earlier
    # (lane recycle). So emit DMAs in kind-homogeneous batches of 8,
    # software-pipelined: a tensor's loads go ~2 batches before its
    # transposes so the casts are done by the time the transpose batch's
    # lane-waits clear.
    def load_batch(src_dram, tag, js):
        tiles = {}
        for j in js:
            lt = ld_p.tile([P, D], F32, name=f"ld_{tag}{j}", tag="ld")
            nc.sync.dma_start(lt[:], src_dram.ap()[j * P : (j + 1) * P, :])
            tiles[j] = lt
        return tiles

    def cast_batch(lds, tag, js, dst_v=None, eng="dve"):
        """fp32->fp16 cast; into c16 staging (for transposes) or straight
        into dst_v[:, j, :] (untransposed weights). eng="act" keeps these
        out of the DVE FIFO when DVE gates PSUM eviction turnaround."""
        cts = {}
        for j in js:
            dst = dst_v[:, j, :] if dst_v is not None else None
            if dst is None:
                ct = c16_p.tile([P, D], F16, name=f"c16_{tag}{j}", tag="c16")
                dst = ct[:]
                cts[j] = ct
            if eng == "act":
                nc.scalar.activation(dst, lds[j][:], AF.Identity)
            else:
                nc.vector.tensor_copy(dst, lds[j][:])
        return cts

    def tr_batch(cts, dst_v, js):
        for j in js:
            nc.sync.dma_start_transpose(dst_v[:, :, j * P : (j + 1) * P], cts[j][:])

    def cast_w_batch(lds, tag, evict_eng):
        """casts for a weight tensor destined for PE transpose: all 8 tiles
        stay resident in w16_p until the transposes consume them."""
        cts = {}
        for j in range(ND):
            ct = w16_p.tile([P, D], F16, name=f"w16_{tag}{j}", tag="w16")
            nc.vector.tensor_copy(ct[:], lds[j][:])
            cts[j] = ct
        return cts

    def transpose_w_pe(cts, dst_v, tag, evict_eng):
        """PE-transpose a weight tensor (head phase: the PE is otherwise
        idle waiting on DMA, so these are free) into dst_v[:, k, :]."""
        for jj in range(2):
            for k2 in range(0, ND, 2):
                psts = [
                    ps_tr.tile([P, 512], F16, name=f"pst_{tag}{u}", tag="pst")
                    for u in range(2)
                ]
                for j4 in range(4):
                    j = jj * 4 + j4
                    for u in range(2):
                        nc.tensor.matmul(
                            psts[u][:, j4 * P : (j4 + 1) * P],
                            cts[j][:, (k2 + u) * P : (k2 + u + 1) * P],
                            identity[:],
                            is_transpose=True,
                            start=(j4 == 0),
                            stop=(j4 == 3),
                        )
                for u in range(2):
                    if evict_eng == "act":
                        nc.scalar.activation(
                            dst_v[:, k2 + u, jj * 512 : (jj + 1) * 512],
                            psts[u][:],
                            AF.Identity,
                        )
                    else:
                        nc.vector.tensor_copy(
                            dst_v[:, k2 + u, jj * 512 : (jj + 1) * 512],
                            psts[u][:],
                        )

    J8 = list(range(8))
    ps_tr = tc.alloc_tile_pool(name="ps_tr", bufs=2, space="PSUM")
    wq_ld = load_batch(wq, "wq", J8)
    wk_ld = load_batch(wk, "wk", J8)
    wq_ct = cast_w_batch(wq_ld, "wq", "act")
    transpose_w_pe(wq_ct, wqta_v, "wq", "act")
    wk_ct = cast_w_batch(wk_ld, "wk", "dve")
    transpose_w_pe(wk_ct, wkta_v, "wk", "dve")
    x0_ld = load_batch(x, "x", list(range(0, 8)))
    x0_ct = cast_batch(x0_ld, "x", list(range(0, 8)))
    tr_batch(x0_ct, xta_v, list(range(0, 8)))

    # ---- G = Wq @ Wk^T ----
    ps_g = tc.alloc_tile_pool(name="ps_g", bufs=2, space="PSUM")
    for m in range(ND):
        psg = ps_g.tile([P, D], F32, name="psg", tag="psg")
        for k in range(ND):
            lhsT = wqta_v[:, k, m * P : (m + 1) * P]
            for h in range(2):
                nc.tensor.matmul(
                    psg[:, h * 512 : (h + 1) * 512],
                    lhsT,
                    wkta_v[:, k, h * 512 : (h + 1) * 512],
                    start=(k == 0),
                    stop=(k == ND - 1),
                )
        nc.vector.tensor_copy(ga_v[:, m, :], psg[:])
    ps_g.release()
    ps_tr.release()

    # ---- HT = G^T @ xT, per 512-wide sq chunk ----
    # remaining pipelined chains: x tail, e, wv. ST's kt loop only needs
    # e-transpose j for kt=j, so the e1 batch never gates the ST start.
    x1_ld = load_batch(x, "x", list(range(8, 16)))
    x1_ct = cast_batch(x1_ld, "x", list(range(8, 16)), eng="act")
    tr_batch(x1_ct, xta_v, list(range(8, 16)))
    e0_ld = load_batch(e, "e", list(range(0, 8)))
    e0_ct = cast_batch(e0_ld, "e", list(range(0, 8)), eng="act")
    tr_batch(e0_ct, eta_v, list(range(0, 8)))
    wv_ld = load_batch(wv, "wv", J8)
    cast_batch(wv_ld, "wv", J8, dst_v=wva_v, eng="act")
    e1_ld = load_batch(e, "e", list(range(8, 16)))
    e1_ct = cast_batch(e1_ld, "e", list(range(8, 16)), eng="act")
    tr_batch(e1_ct, eta_v, list(range(8, 16)))

    ps_proj = tc.alloc_tile_pool(name="ps_proj", bufs=2, space="PSUM")
    for c in range(NC):
        for dot2 in range(0, ND, 2):
            psq = ps_proj.tile([P, D], F32, name="psq", tag="psq")
            for dit in range(ND):
                rhs = xta_v[:, dit, c * 512 : (c + 1) * 512]
                for u in range(2):
                    nc.tensor.matmul(
                        psq[:, u * 512 : (u + 1) * 512],
                        ga_v[:, dit, (dot2 + u) * P : (dot2 + u + 1) * P],
                        rhs,
                        start=(dit == 0),
                        stop=(dit == ND - 1),
                    )
            nc.vector.tensor_copy(
                hta_v[:, dot2 : dot2 + 2, c * 512 : (c + 1) * 512],
                psq[:].rearrange("p (a s) -> p a s", a=2),
            )
    ps_proj.release()

    # left stack teardown (LIFO): everything above eta/hta/wva is dead once
    # the e-chain and HT have drained; releasing here lets pt/v reuse it.
    c16_p.release()
    ld_p.release()
    w16_p.release()
    wkta_p.release()
    wqta_p.release()
    ga_p.release()
    xta_p.release()

    # ---- attention: fused ST + V (shared eT stationaries), exp, then PV ----
    pt_pool = tc.alloc_tile_pool(name="pt", bufs=NSKV, side="right")
    v_pool = tc.alloc_tile_pool(name="v", bufs=NSKV, side="right")

    pT = [pt_pool.tile([P, SQ], BF16, name=f"pT{t}", tag="pT") for t in range(NSKV)]
    vt = [v_pool.tile([P, D], BF16, name=f"v{t}", tag="v") for t in range(NSKV)]
    ps_st = tc.alloc_tile_pool(name="ps_st", bufs=3, space="PSUM")
    ps_v = tc.alloc_tile_pool(name="ps_v", bufs=1, space="PSUM")
    for kt_i in range(NSKV):
        pa = ps_st.tile([P, 1024], F32, name="pss_a", tag="pss_st")
        pb = ps_st.tile([P, 1024], F32, name="pss_b", tag="pss_st")
        psv = ps_v.tile([P, D], F32, name="psv", tag="psv")
        st_tiles = (pa, pa, pb, pb)
        for dit in range(ND):
            lhsT = eta_v[:, dit, kt_i * P : (kt_i + 1) * P]
            first = dit == 0
            last = dit == ND - 1
            for c in range(NC):
                nc.tensor.matmul(
                    st_tiles[c][:, (c % 2) * 512 : (c % 2 + 1) * 512],
                    lhsT,
                    hta_v[:, dit, c * 512 : (c + 1) * 512],
                    start=first,
                    stop=last,
                )
            for h in range(2):
                nc.tensor.matmul(
                    psv[:, h * 512 : (h + 1) * 512],
                    lhsT,
                    wva_v[:, dit, h * 512 : (h + 1) * 512],
                    start=first,
                    stop=last,
                )
        for c in range(NC):
            nc.scalar.activation(
                pT[kt_i][:, c * 512 : (c + 1) * 512],
                st_tiles[c][:, (c % 2) * 512 : (c % 2 + 1) * 512],
                AF.Exp,
                scale=0.125,
            )
        for h in range(2):
            nc.vector.tensor_copy(
                vt[kt_i][:, h * 512 : (h + 1) * 512],
                psv[:, h * 512 : (h + 1) * 512],
            )
    ps_v.release()
    ps_st.release()

    # ---- PV: out = (PT^T @ V) / (PT^T @ 1) ----
    outp = tc.alloc_tile_pool(name="outp", bufs=3, side="right")
    small = tc.alloc_tile_pool(name="small", bufs=4, side="right")
    ps_pv = tc.alloc_tile_pool(name="ps_pv", bufs=2, space="PSUM")
    ps_sum = tc.alloc_tile_pool(name="ps_sum", bufs=1, space="PSUM")
    for sqt in range(NSQ):
        pso = ps_pv.tile([P, D], F32, name="pso", tag="pso")
        psum_s = ps_sum.tile([P, 1], F32, name="psum_s", tag="psum_s")
        for kt_i in range(NSKV):
            lhsT = pT[kt_i][:, sqt * P : (sqt + 1) * P]
            first = kt_i == 0
            last = kt_i == NSKV - 1
            for hh in range(2):
                nc.tensor.matmul(
                    pso[:, hh * 512 : (hh + 1) * 512],
                    lhsT,
                    vt[kt_i][:, hh * 512 : (hh + 1) * 512],
                    start=first,
                    stop=last,
                )
            nc.tensor.matmul(psum_s[:], lhsT, ones_col[:], start=first, stop=last)
        recip = small.tile([P, 1], F32, name="recip", tag="recip")
        nc.vector.reciprocal(recip[:], psum_s[:])
        ot = outp.tile([P, D], F32, name="ot", tag="ot")
        for hh in range(2):
            nc.vector.tensor_scalar_mul(
                ot[:, hh * 512 : (hh + 1) * 512],
                pso[:, hh * 512 : (hh + 1) * 512],
                recip[:],
            )
            nc.sync.dma_start(
                out.ap()[sqt * P : (sqt + 1) * P, hh * 512 : (hh + 1) * 512],
                ot[:, hh * 512 : (hh + 1) * 512],
            )

    ps_sum.release()
    ps_pv.release()
    small.release()
    outp.release()
    v_pool.release()
    pt_pool.release()
    wva_p.release()
    hta_p.release()
    eta_p.release()
    const.release()


def _emit_body(nc, tc, x, e, wq, wk, wv, bq, bk, bv, out):
    """General path (nonzero biases). Unchanged from the baseline kernel."""
    # ---- left-stack pools (released LIFO) ----
    const = tc.alloc_tile_pool(name="const", bufs=1, side="left")
    qt_pool = tc.alloc_tile_pool(name="qt", bufs=ND, side="left")
    kt_pool = tc.alloc_tile_pool(name="kt", bufs=ND, side="left")
    w16_pool = tc.alloc_tile_pool(name="w16", bufs=16, side="left")
    et_pool = tc.alloc_tile_pool(name="et", bufs=ND, side="left")
    ldW = tc.alloc_tile_pool(name="ldW", bufs=3, side="left")
    ldE = tc.alloc_tile_pool(name="ldE", bufs=4, side="left")
    xl16_pool = tc.alloc_tile_pool(name="xl16", bufs=4, side="left")
    ps_proj = tc.alloc_tile_pool(name="ps_proj", bufs=4, space="PSUM")
    ps_tr = tc.alloc_tile_pool(name="ps_tr", bufs=4, space="PSUM")

    def _load_w16(w_dram):
        tiles = []
        for dit in range(ND):
            wl = ldW.tile([P, D], F32, name=f"wl_{w_dram.name}_{dit}", tag="ldw")
            nc.sync.dma_start(wl[:], w_dram.ap()[dit * P : (dit + 1) * P, :])
            w16t = w16_pool.tile([P, D], F16, name=f"w16_{w_dram.name}_{dit}", tag="w16")
            nc.vector.tensor_copy(w16t[:], wl[:])
            tiles.append(w16t)
        return tiles

    identity = const.tile([P, P], F16, tag="ident")
    make_identity(nc, identity[:])
    ones_row = const.tile([1, P], F16, tag="ones_row")
    nc.gpsimd.memset(ones_row[:], 1.0)
    ones_col = const.tile([P, 1], BF16, tag="ones_col")
    nc.gpsimd.memset(ones_col[:], 1.0)
    bqt = const.tile([P, ND], F32, tag="bqt")
    nc.sync.dma_start(bqt[:], bq.ap().rearrange("(t p) -> p t", p=P))
    bkt = const.tile([P, ND], F32, tag="bkt")
    nc.sync.dma_start(bkt[:], bk.ap().rearrange("(t p) -> p t", p=P))
    bvl = ldW.tile([1, D], F32, tag="ldw")
    nc.sync.dma_start(bvl[:], bv.ap().rearrange("(a n) -> a n", a=1))
    bv16 = const.tile([1, D], F16, tag="bv16")
    nc.vector.tensor_copy(bv16[:], bvl[:])

    def transpose_group(ld_tiles, dst_write, tag):
        l16 = []
        for j in range(4):
            t16 = xl16_pool.tile([P, D], F16, name=f"l16_{tag}_{j}", tag="l16")
            nc.vector.tensor_copy(t16[:], ld_tiles[j][:])
            l16.append(t16)
        for dit2 in range(0, ND, 2):
            psts = [
                ps_tr.tile([P, 512], F16, name=f"pst_{tag}{u}", tag="pst")
                for u in range(2)
            ]
            for j in range(4):
                for u in range(2):
                    nc.tensor.matmul(
                        psts[u][:, j * P : (j + 1) * P],
                        l16[j][:, (dit2 + u) * P : (dit2 + u + 1) * P],
                        identity[:],
                        is_transpose=True,
                        start=(j == 0),
                        stop=(j == 3),
                    )
            for u in range(2):
                dst_write(dit2 + u, psts[u])

    def project_chunk(w16, rhs_of_dit, dst_tiles, bias_cols, c):
        for dot2 in range(0, ND, 2):
            psq = [
                ps_proj.tile([P, 512], F32, name=f"psq{u}", tag="psp")
                for u in range(2)
            ]
            for dit in range(ND):
                for u in range(2):
                    nc.tensor.matmul(
                        psq[u][:],
                        w16[dit][:, (dot2 + u) * P : (dot2 + u + 1) * P],
                        rhs_of_dit(dit),
                        start=(dit == 0),
                        stop=(dit == ND - 1),
                    )
            for u in range(2):
                nc.scalar.activation(
                    dst_tiles[dot2 + u][:, c * 512 : (c + 1) * 512],
                    psq[u][:],
                    AF.Identity,
                    bias=bias_cols[:, dot2 + u : dot2 + u + 1],
                )

    # ---- x -> xT chunks -> QT, interleaved per 512-chunk ----
    xtc_pool = tc.alloc_tile_pool(name="xtc", bufs=2 * ND, side="left")
    ldX = tc.alloc_tile_pool(name="ldX", bufs=4, side="left")

    def load_group(pool, src_dram, c, tag):
        tiles = []
        for j in range(4):
            st = c * 4 + j
            t = pool.tile([P, D], F32, name=f"{tag}{c}_{j}", tag=tag)
            nc.sync.dma_start(t[:], src_dram.ap()[st * P : (st + 1) * P, :])
            tiles.append(t)
        return tiles

    qT = [qt_pool.tile([P, SQ], F16, name=f"qT{d}", tag="qT") for d in range(ND)]
    xg = {0: load_group(ldX, x, 0, "ldx")}
    wq16 = _load_w16(wq)
    eg = {0: load_group(ldE, e, 0, "lde")}
    for c in range(NC):
        if c + 1 < NC:
            xg[c + 1] = load_group(ldX, x, c + 1, "ldx")
        xtc = [
            xtc_pool.tile([P, 512], F16, name=f"xtc{c}_{d}", tag="xtc")
            for d in range(ND)
        ]

        def wr_x(dit, pst, xtc=xtc):
            nc.vector.tensor_copy(xtc[dit][:], pst[:])

        transpose_group(xg.pop(c), wr_x, "x")
        project_chunk(wq16, lambda dit, xtc=xtc: xtc[dit][:], qT, bqt, c)
    ldX.release()
    xtc_pool.release()

    # ---- e -> eT (kept resident) -> KT, interleaved per 512-chunk ----
    eT = [et_pool.tile([P, SKV], F16, name=f"eT{d}", tag="eT") for d in range(ND)]
    kT = [kt_pool.tile([P, SKV], F16, name=f"kT{d}", tag="kT") for d in range(ND)]
    wk16 = _load_w16(wk)
    for c in range(NC):
        if c + 1 < NC:
            eg[c + 1] = load_group(ldE, e, c + 1, "lde")

        def wr_e(dit, pst, c=c):
            nc.vector.tensor_copy(eT[dit][:, c * 512 : (c + 1) * 512], pst[:])

        transpose_group(eg.pop(c), wr_e, "e")
        project_chunk(
            wk16,
            lambda dit, c=c: eT[dit][:, c * 512 : (c + 1) * 512],
            kT,
            bkt,
            c,
        )
    xl16_pool.release()
    ldE.release()

    # ---- Wv ; V ----
    wv16 = _load_w16(wv)
    ldW.release()
    ps_tr.release()

    v_pool = tc.alloc_tile_pool(name="v", bufs=NSKV, side="right")
    vt = [v_pool.tile([P, D], BF16, name=f"v{t}", tag="v") for t in range(NSKV)]
    for kt_i in range(NSKV):
        ps_half = []
        for h in range(2):
            psv = ps_proj.tile([P, 512], F32, name=f"psv{h}", tag="psp")
            nc.tensor.matmul(
                psv[:],
                ones_row[:],
                bv16[:, h * 512 : (h + 1) * 512],
                start=True,
                stop=False,
            )
            ps_half.append(psv)
        for dit in range(ND):
            for h in range(2):
                nc.tensor.matmul(
                    ps_half[h][:],
                    eT[dit][:, kt_i * P : (kt_i + 1) * P],
                    wv16[dit][:, h * 512 : (h + 1) * 512],
                    start=False,
                    stop=(dit == ND - 1),
                )
        for h in range(2):
            nc.vector.tensor_copy(vt[kt_i][:, h * 512 : (h + 1) * 512], ps_half[h][:])

    ps_proj.release()
    et_pool.release()
    w16_pool.release()

    # ---- attention: ST+exp phase (full PT materialized), then PV phase ----
    pt_pool = tc.alloc_tile_pool(name="pt", bufs=NSKV, side="right")
    outp = tc.alloc_tile_pool(name="outp", bufs=3, side="right")
    small = tc.alloc_tile_pool(name="small", bufs=4, side="right")

    pT = [pt_pool.tile([P, SQ], BF16, name=f"pT{t}", tag="pT") for t in range(NSKV)]
    ps_st = tc.alloc_tile_pool(name="ps_st", bufs=2, space="PSUM")
    for kt_i in range(NSKV):
        pss = ps_st.tile([P, SQ], F32, name="pss_st", tag="pss_st")
        for dit in range(ND):
            lhsT = kT[dit][:, kt_i * P : (kt_i + 1) * P]
            for c in range(NC):
                nc.tensor.matmul(
                    pss[:, c * 512 : (c + 1) * 512],
                    lhsT,
                    qT[dit][:, c * 512 : (c + 1) * 512],
                    start=(dit == 0),
                    stop=(dit == ND - 1),
                )
        for c in range(NC):
            nc.scalar.activation(
                pT[kt_i][:, c * 512 : (c + 1) * 512],
                pss[:, c * 512 : (c + 1) * 512],
                AF.Exp,
                scale=0.125,
            )
    ps_st.release()

    ps_pv = tc.alloc_tile_pool(name="ps_pv", bufs=2, space="PSUM")
    ps_sum = tc.alloc_tile_pool(name="ps_sum", bufs=2, space="PSUM")
    for sqt in range(NSQ):
        pso = ps_pv.tile([P, D], F32, name="pso", tag="pso")
        psum_s = ps_sum.tile([P, 1], F32, name="psum_s", tag="psum_s")
        for kt_i in range(NSKV):
            lhsT = pT[kt_i][:, sqt * P : (sqt + 1) * P]
            first = kt_i == 0
            last = kt_i == NSKV - 1
            for h in range(2):
                nc.tensor.matmul(
                    pso[:, h * 512 : (h + 1) * 512],
                    lhsT,
                    vt[kt_i][:, h * 512 : (h + 1) * 512],
                    start=first,
                    stop=last,
                )
            nc.tensor.matmul(psum_s[:], lhsT, ones_col[:], start=first, stop=last)
        recip = small.tile([P, 1], F32, name="recip", tag="recip")
        nc.vector.reciprocal(recip[:], psum_s[:])
        ot = outp.tile([P, D], F32, name="ot", tag="ot")
        nc.vector.tensor_scalar_mul(ot[:], pso[:], recip[:])
        nc.sync.dma_start(out.ap()[sqt * P : (sqt + 1) * P, :], ot[:])

    ps_sum.release()
    ps_pv.release()
    small.release()
    outp.release()
    pt_pool.release()
    v_pool.release()
    kt_pool.release()
    qt_pool.release()
    const.release()


_NC_CACHE = {}


def _get_nc(fast):
    if fast not in _NC_CACHE:
        _NC_CACHE[fast] = build(fast=fast)
    return _NC_CACHE[fast]


def kernel(
    hidden_states,
    encoder_hidden_states,
    Wq,
    bq,
    Wk,
    bk,
    Wv,
    bv,
    _trace=False,
    _trace_kwargs=None,
):
    hs = np.ascontiguousarray(np.asarray(hidden_states, np.float32))
    es = np.ascontiguousarray(np.asarray(encoder_hidden_states, np.float32))
    wq_ = np.ascontiguousarray(np.asarray(Wq, np.float32))
    wk_ = np.ascontiguousarray(np.asarray(Wk, np.float32))
    wv_ = np.ascontiguousarray(np.asarray(Wv, np.float32))
    bq_ = np.ascontiguousarray(np.asarray(bq, np.float32))
    bk_ = np.ascontiguousarray(np.asarray(bk, np.float32))
    bv_ = np.ascontiguousarray(np.asarray(bv, np.float32))

    # The S = x (Wq Wk^T) e^T association only absorbs the biases when they
    # are zero; fall back to the general module otherwise.
    fast = not (bq_.any() or bk_.any() or bv_.any())
    nc = _get_nc(fast)
    in_maps = [
        {
            "x": hs[c],
            "e": es[c],
            "wq": wq_,
            "wk": wk_,
            "wv": wv_,
            "bq": bq_,
            "bk": bk_,
            "bv": bv_,
        }
        for c in range(N_CORES)
    ]
    res = run_bass_kernel_spmd(
        nc,
        in_maps,
        list(range(N_CORES)),
        trace=_trace,
        **(_trace_kwargs or {}),
    )
    out = np.stack([res.results[c]["out"] for c in range(N_CORES)], axis=0)
    if _trace:
        return out, res
    return out


# revision 31
# speedup vs baseline: 1.1021x; 1.0324x over previous
"""Cross-attention kernel for Trainium2, data-parallel over batch on 8 cores.

Per core (one batch element):
  Q = x @ Wq + bq ; K = e @ Wk + bk ; V = e @ Wv + bv
  out = softmax(Q K^T / 8) @ V

Fast path (biases all zero, which is what setup_inputs produces) uses the
association S = x (Wq Wk^T) e^T:
  G  = Wq @ Wk^T          [d_in, d_in]   one 1024^3 GEMM, replaces K-proj
  HT = G^T @ xT           [d_in_e, sq]
  ST = eT^T @ HT          [skv, sq]      fused with V = e @ Wv (shared eT
                                         stationary operand)
  PT = exp(ST/8)          bf16           (no max subtraction: |s/8| < ~25)
  out = (PT^T @ V) * 1/(PT^T @ 1)

All input transposes (x, e, Wq, Wk) run on the DMA XBAR engine
(dma_start_transpose, 2-byte dtype) instead of the PE, so the tensor engine
only executes the five GEMM phases. Inputs are loaded fp32, cast to fp16 on
DVE, then XBAR-transposed into flat [128, 8*S] layouts:
  xta[p, d, s] = x[s, d*128+p]   (view [128, 8, 2048])
  eta          likewise for e
  wqta[p, a, c] = wq[c, a*128+p] (view [128, 8, 1024]), wkta likewise

Numerics: fp16 projections + bf16 probs/V gives ~2e-3 rel L2 error vs the
fp32 reference (bf16 everywhere would be ~1e-2).
"""

import numpy as np

import concourse.bacc as bacc
import concourse.bass as bass
import concourse.mybir as mybir
import concourse.tile as tile
from concourse.bass_utils import run_bass_kernel_spmd
from concourse.masks import make_identity

P = 128
D = 1024
ND = D // P  # 8 d tiles
SQ = 2048
NSQ = SQ // P  # 16
SKV = 2048
NSKV = SKV // P  # 16
NC = SQ // 512  # 4 strips of 512 along s
N_CORES = 8

F32 = mybir.dt.float32
F16 = mybir.dt.float16
BF16 = mybir.dt.bfloat16
AF = mybir.ActivationFunctionType


def build(reps=1, fast=False):
    nc = bacc.Bacc("TRN2", target_bir_lowering=False, debug=False)
    x = nc.declare_dram_parameter("x", [SQ, D], F32, isOutput=False)
    e = nc.declare_dram_parameter("e", [SKV, D], F32, isOutput=False)
    wq = nc.declare_dram_parameter("wq", [D, D], F32, isOutput=False)
    wk = nc.declare_dram_parameter("wk", [D, D], F32, isOutput=False)
    wv = nc.declare_dram_parameter("wv", [D, D], F32, isOutput=False)
    bq = nc.declare_dram_parameter("bq", [D], F32, isOutput=False)
    bk = nc.declare_dram_parameter("bk", [D], F32, isOutput=False)
    bv = nc.declare_dram_parameter("bv", [D], F32, isOutput=False)
    out = nc.declare_dram_parameter("out", [SQ, D], F32, isOutput=True)

    with tile.TileContext(nc) as tc:
        for _rep in range(reps):
            if fast:
                _emit_body_fast(nc, tc, x, e, wq, wk, wv, out)
            else:
                _emit_body(nc, tc, x, e, wq, wk, wv, bq, bk, bv, out)

    nc.compile()
    return nc


def _emit_body_fast(nc, tc, x, e, wq, wk, wv, out):
    # ---- left-stack pools (released LIFO) ----
    const = tc.alloc_tile_pool(name="const", bufs=1, side="left")
    eta_p = tc.alloc_tile_pool(name="eta", bufs=1, side="left")
    hta_p = tc.alloc_tile_pool(name="hta", bufs=1, side="left")
    wva_p = tc.alloc_tile_pool(name="wva", bufs=1, side="left")
    xta_p = tc.alloc_tile_pool(name="xta", bufs=1, side="left")
    ga_p = tc.alloc_tile_pool(name="ga", bufs=1, side="left")
    wqta_p = tc.alloc_tile_pool(name="wqta", bufs=1, side="left")
    wkta_p = tc.alloc_tile_pool(name="wkta", bufs=1, side="left")
    w16_p = tc.alloc_tile_pool(name="w16", bufs=ND, side="left")
    ld_p = tc.alloc_tile_pool(name="ld", bufs=5, side="left")
    c16_p = tc.alloc_tile_pool(name="c16", bufs=5, side="left")

    ones_col = const.tile([P, 1], BF16, tag="ones_col")
    nc.gpsimd.memset(ones_col[:], 1.0)
    identity = const.tile([P, P], F16, tag="ident")
    make_identity(nc, identity[:])
    one_f32 = const.tile([1, 1], F32, tag="one_f32")
    nc.gpsimd.memset(one_f32[:], 1.0)

    # flat transposed layouts; [p, t, s] views
    eta = eta_p.tile([P, ND * SKV], F16, tag="eta")
    eta_v = eta[:].rearrange("p (a s) -> p a s", a=ND)
    hta = hta_p.tile([P, ND * SQ], F16, tag="hta")
    hta_v = hta[:].rearrange("p (a s) -> p a s", a=ND)
    wva = wva_p.tile([P, ND * D], F16, tag="wva")
    wva_v = wva[:].rearrange("p (a s) -> p a s", a=ND)
    xta = xta_p.tile([P, ND * SQ], F16, tag="xta")
    xta_v = xta[:].rearrange("p (a s) -> p a s", a=ND)
    ga = ga_p.tile([P, ND * D], F16, tag="ga")
    ga_v = ga[:].rearrange("p (a s) -> p a s", a=ND)
    wqta = wqta_p.tile([P, ND * D], F16, tag="wqta")
    wqta_v = wqta[:].rearrange("p (a s) -> p a s", a=ND)
    wkta = wkta_p.tile([P, ND * D], F16, tag="wkta")
    wkta_v = wkta[:].rearrange("p (a s) -> p a s", a=ND)

    # ---- input chains ----
    # The tile framework assigns DMA ops to the 8 HWDGE lanes round-robin in
    # emission order; a DMA waits for completion of the DMA 8 slots earlier
    # (lane recycle). So emit DMAs in kind-homogeneous batches of 8,
    # software-pipelined: a tensor's loads go ~2 batches before its
    # transposes so the casts are done by the time the transpose batch's
    # lane-waits clear.
    def load_batch(src_dram, tag, js):
        tiles = {}
        for j in js:
            lt = ld_p.tile([P, D], F32, name=f"ld_{tag}{j}", tag="ld")
            nc.sync.dma_start(lt[:], src_dram.ap()[j * P : (j + 1) * P, :])
            tiles[j] = lt
        return tiles

    def cast_batch(lds, tag, js, dst_v=None, eng="dve"):
        """fp32->fp16 cast; into c16 staging (for transposes) or straight
        into dst_v[:, j, :] (untransposed weights). eng="act" keeps these
        out of the DVE FIFO when DVE gates PSUM eviction turnaround."""
        cts = {}
        for j in js:
            dst = dst_v[:, j, :] if dst_v is not None else None
            if dst is None:
                ct = c16_p.tile([P, D], F16, name=f"c16_{tag}{j}", tag="c16")
                dst = ct[:]
                cts[j] = ct
            if eng == "act":
                nc.scalar.activation(dst, lds[j][:], AF.Identity)
            else:
                nc.vector.tensor_copy(dst, lds[j][:])
        return cts

    def tr_batch(cts, dst_v, js):
        for j in js:
            nc.sync.dma_start_transpose(dst_v[:, :, j * P : (j + 1) * P], cts[j][:])

    def cast_w_batch(lds, tag, evict_eng):
        """casts for a weight tensor destined for PE transpose: all 8 tiles
        stay resident in w16_p until the transposes consume them."""
        cts = {}
        for j in range(ND):
            ct = w16_p.tile([P, D], F16, name=f"w16_{tag}{j}", tag="w16")
            nc.vector.tensor_copy(ct[:], lds[j][:])
            cts[j] = ct
        return cts

    def transpose_w_pe(cts, dst_v, tag, evict_eng):
        """PE-transpose a weight tensor (head phase: the PE is otherwise
        idle waiting on DMA, so these are free) into dst_v[:, k, :]."""
        for jj in range(2):
            for k2 in range(0, ND, 2):
                psts = [
                    ps_tr.tile([P, 512], F16, name=f"pst_{tag}{u}", tag="pst")
                    for u in range(2)
                ]
                for j4 in range(4):
                    j = jj * 4 + j4
                    for u in range(2):
                        nc.tensor.matmul(
                            psts[u][:, j4 * P : (j4 + 1) * P],
                            cts[j][:, (k2 + u) * P : (k2 + u + 1) * P],
                            identity[:],
                            is_transpose=True,
                            start=(j4 == 0),
                            stop=(j4 == 3),
                        )
                for u in range(2):
                    if evict_eng == "act":
                        nc.scalar.activation(
                            dst_v[:, k2 + u, jj * 512 : (jj + 1) * 512],
                            psts[u][:],
                            AF.Identity,
                        )
                    else:
                        nc.vector.tensor_copy(
                            dst_v[:, k2 + u, jj * 512 : (jj + 1) * 512],
                            psts[u][:],
                        )

    J8 = list(range(8))
    ps_tr = tc.alloc_tile_pool(name="ps_tr", bufs=2, space="PSUM")
    wq_ld = load_batch(wq, "wq", J8)
    wk_ld = load_batch(wk, "wk", J8)
    wq_ct = cast_w_batch(wq_ld, "wq", "act")
    transpose_w_pe(wq_ct, wqta_v, "wq", "act")
    wk_ct = cast_w_batch(wk_ld, "wk", "dve")
    transpose_w_pe(wk_ct, wkta_v, "wk", "dve")
    x0_ld = load_batch(x, "x", list(range(0, 8)))
    x0_ct = cast_batch(x0_ld, "x", list(range(0, 8)))
    tr_batch(x0_ct, xta_v, list(range(0, 8)))

    # ---- G = Wq @ Wk^T ----
    ps_g = tc.alloc_tile_pool(name="ps_g", bufs=2, space="PSUM")
    for m in range(ND):
        psg = ps_g.tile([P, D], F32, name="psg", tag="psg")
        for k in range(ND):
            lhsT = wqta_v[:, k, m * P : (m + 1) * P]
            for h in range(2):
                nc.tensor.matmul(
                    psg[:, h * 512 : (h + 1) * 512],
                    lhsT,
                    wkta_v[:, k, h * 512 : (h + 1) * 512],
                    start=(k == 0),
                    stop=(k == ND - 1),
                )
        nc.vector.tensor_copy(ga_v[:, m, :], psg[:])
    ps_g.release()
    ps_tr.release()

    # ---- HT = G^T @ xT, per 512-wide sq chunk ----
    # remaining pipelined chains: x tail, e, wv. ST's kt loop only needs
    # e-transpose j for kt=j, so the e1 batch never gates the ST start.
    x1_ld = load_batch(x, "x", list(range(8, 16)))
    x1_ct = cast_batch(x1_ld, "x", list(range(8, 16)))
    tr_batch(x1_ct, xta_v, list(range(8, 16)))
    e0_ld = load_batch(e, "e", list(range(0, 8)))
    e0_ct = cast_batch(e0_ld, "e", list(range(0, 8)))
    tr_batch(e0_ct, eta_v, list(range(0, 8)))
    wv_ld = load_batch(wv, "wv", J8)
    cast_batch(wv_ld, "wv", J8, dst_v=wva_v, eng="act")
    e1_ld = load_batch(e, "e", list(range(8, 16)))
    e1_ct = cast_batch(e1_ld, "e", list(range(8, 16)))
    tr_batch(e1_ct, eta_v, list(range(8, 16)))

    ps_proj = tc.alloc_tile_pool(name="ps_proj", bufs=2, space="PSUM")
    # Two passes of two 512-chunks each (pass 0 starts as soon as x[0:8] is
    # transposed). Within a pass the stationary ga slice is shared by the two
    # chunk matmuls (LDW reuse), writing the two halves of one [P,1024] psum.
    for cp in range(2):
        for dot in range(ND):
            psq = ps_proj.tile([P, D], F32, name="psq", tag="psq")
            for dit in range(ND):
                lhsT = ga_v[:, dit, dot * P : (dot + 1) * P]
                for u in range(2):
                    c = 2 * cp + u
                    nc.tensor.matmul(
                        psq[:, u * 512 : (u + 1) * 512],
                        lhsT,
                        xta_v[:, dit, c * 512 : (c + 1) * 512],
                        start=(dit == 0),
                        stop=(dit == ND - 1),
                    )
            nc.vector.tensor_copy(
                hta_v[:, dot, 2 * cp * 512 : (2 * cp + 2) * 512],
                psq[:],
            )
    ps_proj.release()

    # left stack teardown (LIFO): everything above eta/hta/wva is dead once
    # the e-chain and HT have drained; releasing here lets pt/v reuse it.
    c16_p.release()
    ld_p.release()
    w16_p.release()
    wkta_p.release()
    wqta_p.release()
    ga_p.release()
    xta_p.release()

    # ---- V = e @ Wv (own phase: PV-like pattern, fully double-buffered) ----
    pt_pool = tc.alloc_tile_pool(name="pt", bufs=NSKV, side="right")
    v_pool = tc.alloc_tile_pool(name="v", bufs=NSKV, side="right")

    pT = [pt_pool.tile([P, SQ], BF16, name=f"pT{t}", tag="pT") for t in range(NSKV)]
    vt = [v_pool.tile([P, D], BF16, name=f"v{t}", tag="v") for t in range(NSKV)]
    ps_vp = tc.alloc_tile_pool(name="ps_vp", bufs=2, space="PSUM")
    for kt_i in range(NSKV):
        psv = ps_vp.tile([P, D], F32, name="psv", tag="psv")
        for dit in range(ND):
            lhsT = eta_v[:, dit, kt_i * P : (kt_i + 1) * P]
            for h in range(2):
                nc.tensor.matmul(
                    psv[:, h * 512 : (h + 1) * 512],
                    lhsT,
                    wva_v[:, dit, h * 512 : (h + 1) * 512],
                    start=(dit == 0),
                    stop=(dit == ND - 1),
                )
        nc.vector.tensor_copy(vt[kt_i][:], psv[:])
    ps_vp.release()

    # ---- ST = eT^T @ HT, exp -> PT. Four [P,1024] psum bufs: kt+1's pa/pb
    # are fully disjoint from kt's, so the exp evictions have a whole kt of
    # slack (HW stalls hard without this). ----
    ps_st = tc.alloc_tile_pool(name="ps_st", bufs=4, space="PSUM")
    for kt_i in range(NSKV):
        pa = ps_st.tile([P, 1024], F32, name="pss_a", tag="pss_st")
        pb = ps_st.tile([P, 1024], F32, name="pss_b", tag="pss_st")
        st_tiles = (pa, pa, pb, pb)
        for dit in range(ND):
            lhsT = eta_v[:, dit, kt_i * P : (kt_i + 1) * P]
            first = dit == 0
            last = dit == ND - 1
            for c in range(NC):
                nc.tensor.matmul(
                    st_tiles[c][:, (c % 2) * 512 : (c % 2 + 1) * 512],
                    lhsT,
                    hta_v[:, dit, c * 512 : (c + 1) * 512],
                    start=first,
                    stop=last,
                )
        nc.scalar.activation(pT[kt_i][:, 0:1024], pa[:], AF.Exp, scale=0.125)
        nc.scalar.activation(pT[kt_i][:, 1024:2048], pb[:], AF.Exp, scale=0.125)
    ps_st.release()

    # ---- PV: out = (PT^T @ V) / (PT^T @ 1) ----
    outp = tc.alloc_tile_pool(name="outp", bufs=3, side="right")
    small = tc.alloc_tile_pool(name="small", bufs=4, side="right")
    ps_pv = tc.alloc_tile_pool(name="ps_pv", bufs=2, space="PSUM")
    ps_sum = tc.alloc_tile_pool(name="ps_sum", bufs=2, space="PSUM")
    for sqt in range(NSQ):
        pso = ps_pv.tile([P, D], F32, name="pso", tag="pso")
        psum_s = ps_sum.tile([P, 1], F32, name="psum_s", tag="psum_s")
        for kt_i in range(NSKV):
            lhsT = pT[kt_i][:, sqt * P : (sqt + 1) * P]
            first = kt_i == 0
            last = kt_i == NSKV - 1
            for hh in range(2):
                nc.tensor.matmul(
                    pso[:, hh * 512 : (hh + 1) * 512],
                    lhsT,
                    vt[kt_i][:, hh * 512 : (hh + 1) * 512],
                    start=first,
                    stop=last,
                )
            nc.tensor.matmul(psum_s[:], lhsT, ones_col[:], start=first, stop=last)
        recip = small.tile([P, 1], F32, name="recip", tag="recip")
        nc.vector.reciprocal(recip[:], psum_s[:])
        ot = outp.tile([P, D], F32, name="ot", tag="ot")
        for hh in range(2):
            nc.vector.tensor_scalar_mul(
                ot[:, hh * 512 : (hh + 1) * 512],
                pso[:, hh * 512 : (hh + 1) * 512],
                recip[:],
            )
            nc.sync.dma_start(
                out.ap()[sqt * P : (sqt + 1) * P, hh * 512 : (hh + 1) * 512],
                ot[:, hh * 512 : (hh + 1) * 512],
            )

    ps_sum.release()
    ps_pv.release()
    small.release()
    outp.release()
    v_pool.release()
    pt_pool.release()
    wva_p.release()
    hta_p.release()
    eta_p.release()
    const.release()


def _emit_body(nc, tc, x, e, wq, wk, wv, bq, bk, bv, out):
    """General path (nonzero biases). Unchanged from the baseline kernel."""
    # ---- left-stack pools (released LIFO) ----
    const = tc.alloc_tile_pool(name="const", bufs=1, side="left")
    qt_pool = tc.alloc_tile_pool(name="qt", bufs=ND, side="left")
    kt_pool = tc.alloc_tile_pool(name="kt", bufs=ND, side="left")
    w16_pool = tc.alloc_tile_pool(name="w16", bufs=16, side="left")
    et_pool = tc.alloc_tile_pool(name="et", bufs=ND, side="left")
    ldW = tc.alloc_tile_pool(name="ldW", bufs=3, side="left")
    ldE = tc.alloc_tile_pool(name="ldE", bufs=4, side="left")
    xl16_pool = tc.alloc_tile_pool(name="xl16", bufs=4, side="left")
    ps_proj = tc.alloc_tile_pool(name="ps_proj", bufs=4, space="PSUM")
    ps_tr = tc.alloc_tile_pool(name="ps_tr", bufs=4, space="PSUM")

    def _load_w16(w_dram):
        tiles = []
        for dit in range(ND):
            wl = ldW.tile([P, D], F32, name=f"wl_{w_dram.name}_{dit}", tag="ldw")
            nc.sync.dma_start(wl[:], w_dram.ap()[dit * P : (dit + 1) * P, :])
            w16t = w16_pool.tile([P, D], F16, name=f"w16_{w_dram.name}_{dit}", tag="w16")
            nc.vector.tensor_copy(w16t[:], wl[:])
            tiles.append(w16t)
        return tiles

    identity = const.tile([P, P], F16, tag="ident")
    make_identity(nc, identity[:])
    ones_row = const.tile([1, P], F16, tag="ones_row")
    nc.gpsimd.memset(ones_row[:], 1.0)
    ones_col = const.tile([P, 1], BF16, tag="ones_col")
    nc.gpsimd.memset(ones_col[:], 1.0)
    bqt = const.tile([P, ND], F32, tag="bqt")
    nc.sync.dma_start(bqt[:], bq.ap().rearrange("(t p) -> p t", p=P))
    bkt = const.tile([P, ND], F32, tag="bkt")
    nc.sync.dma_start(bkt[:], bk.ap().rearrange("(t p) -> p t", p=P))
    bvl = ldW.tile([1, D], F32, tag="ldw")
    nc.sync.dma_start(bvl[:], bv.ap().rearrange("(a n) -> a n", a=1))
    bv16 = const.tile([1, D], F16, tag="bv16")
    nc.vector.tensor_copy(bv16[:], bvl[:])

    def transpose_group(ld_tiles, dst_write, tag):
        l16 = []
        for j in range(4):
            t16 = xl16_pool.tile([P, D], F16, name=f"l16_{tag}_{j}", tag="l16")
            nc.vector.tensor_copy(t16[:], ld_tiles[j][:])
            l16.append(t16)
        for dit2 in range(0, ND, 2):
            psts = [
                ps_tr.tile([P, 512], F16, name=f"pst_{tag}{u}", tag="pst")
                for u in range(2)
            ]
            for j in range(4):
                for u in range(2):
                    nc.tensor.matmul(
                        psts[u][:, j * P : (j + 1) * P],
                        l16[j][:, (dit2 + u) * P : (dit2 + u + 1) * P],
                        identity[:],
                        is_transpose=True,
                        start=(j == 0),
                        stop=(j == 3),
                    )
            for u in range(2):
                dst_write(dit2 + u, psts[u])

    def project_chunk(w16, rhs_of_dit, dst_tiles, bias_cols, c):
        for dot2 in range(0, ND, 2):
            psq = [
                ps_proj.tile([P, 512], F32, name=f"psq{u}", tag="psp")
                for u in range(2)
            ]
            for dit in range(ND):
                for u in range(2):
                    nc.tensor.matmul(
                        psq[u][:],
                        w16[dit][:, (dot2 + u) * P : (dot2 + u + 1) * P],
                        rhs_of_dit(dit),
                        start=(dit == 0),
                        stop=(dit == ND - 1),
                    )
            for u in range(2):
                nc.scalar.activation(
                    dst_tiles[dot2 + u][:, c * 512 : (c + 1) * 512],
                    psq[u][:],
                    AF.Identity,
                    bias=bias_cols[:, dot2 + u : dot2 + u + 1],
                )

    # ---- x -> xT chunks -> QT, interleaved per 512-chunk ----
    xtc_pool = tc.alloc_tile_pool(name="xtc", bufs=2 * ND, side="left")
    ldX = tc.alloc_tile_pool(name="ldX", bufs=4, side="left")

    def load_group(pool, src_dram, c, tag):
        tiles = []
        for j in range(4):
            st = c * 4 + j
            t = pool.tile([P, D], F32, name=f"{tag}{c}_{j}", tag=tag)
            nc.sync.dma_start(t[:], src_dram.ap()[st * P : (st + 1) * P, :])
            tiles.append(t)
        return tiles

    qT = [qt_pool.tile([P, SQ], F16, name=f"qT{d}", tag="qT") for d in range(ND)]
    xg = {0: load_group(ldX, x, 0, "ldx")}
    wq16 = _load_w16(wq)
    eg = {0: load_group(ldE, e, 0, "lde")}
    for c in range(NC):
        if c + 1 < NC:
            xg[c + 1] = load_group(ldX, x, c + 1, "ldx")
        xtc = [
            xtc_pool.tile([P, 512], F16, name=f"xtc{c}_{d}", tag="xtc")
            for d in range(ND)
        ]

        def wr_x(dit, pst, xtc=xtc):
            nc.vector.tensor_copy(xtc[dit][:], pst[:])

        transpose_group(xg.pop(c), wr_x, "x")
        project_chunk(wq16, lambda dit, xtc=xtc: xtc[dit][:], qT, bqt, c)
    ldX.release()
    xtc_pool.release()

    # ---- e -> eT (kept resident) -> KT, interleaved per 512-chunk ----
    eT = [et_pool.tile([P, SKV], F16, name=f"eT{d}", tag="eT") for d in range(ND)]
    kT = [kt_pool.tile([P, SKV], F16, name=f"kT{d}", tag="kT") for d in range(ND)]
    wk16 = _load_w16(wk)
    for c in range(NC):
        if c + 1 < NC:
            eg[c + 1] = load_group(ldE, e, c + 1, "lde")

        def wr_e(dit, pst, c=c):
            nc.vector.tensor_copy(eT[dit][:, c * 512 : (c + 1) * 512], pst[:])

        transpose_group(eg.pop(c), wr_e, "e")
        project_chunk(
            wk16,
            lambda dit, c=c: eT[dit][:, c * 512 : (c + 1) * 512],
            kT,
            bkt,
            c,
        )
    xl16_pool.release()
    ldE.release()

    # ---- Wv ; V ----
    wv16 = _load_w16(wv)
    ldW.release()
    ps_tr.release()

    v_pool = tc.alloc_tile_pool(name="v", bufs=NSKV, side="right")
    vt = [v_pool.tile([P, D], BF16, name=f"v{t}", tag="v") for t in range(NSKV)]
    for kt_i in range(NSKV):
        ps_half = []
        for h in range(2):
            psv = ps_proj.tile([P, 512], F32, name=f"psv{h}", tag="psp")
            nc.tensor.matmul(
                psv[:],
                ones_row[:],
                bv16[:, h * 512 : (h + 1) * 512],
                start=True,
                stop=False,
            )
            ps_half.append(psv)
        for dit in range(ND):
            for h in range(2):
                nc.tensor.matmul(
                    ps_half[h][:],
                    eT[dit][:, kt_i * P : (kt_i + 1) * P],
                    wv16[dit][:, h * 512 : (h + 1) * 512],
                    start=False,
                    stop=(dit == ND - 1),
                )
        for h in range(2):
            nc.vector.tensor_copy(vt[kt_i][:, h * 512 : (h + 1) * 512], ps_half[h][:])

    ps_proj.release()
    et_pool.release()
    w16_pool.release()

    # ---- attention: ST+exp phase (full PT materialized), then PV phase ----
    pt_pool = tc.alloc_tile_pool(name="pt", bufs=NSKV, side="right")
    outp = tc.alloc_tile_pool(name="outp", bufs=3, side="right")
    small = tc.alloc_tile_pool(name="small", bufs=4, side="right")

    pT = [pt_pool.tile([P, SQ], BF16, name=f"pT{t}", tag="pT") for t in range(NSKV)]
    ps_st = tc.alloc_tile_pool(name="ps_st", bufs=2, space="PSUM")
    for kt_i in range(NSKV):
        pss = ps_st.tile([P, SQ], F32, name="pss_st", tag="pss_st")
        for dit in range(ND):
            lhsT = kT[dit][:, kt_i * P : (kt_i + 1) * P]
            for c in range(NC):
                nc.tensor.matmul(
                    pss[:, c * 512 : (c + 1) * 512],
                    lhsT,
                    qT[dit][:, c * 512 : (c + 1) * 512],
                    start=(dit == 0),
                    stop=(dit == ND - 1),
                )
        for c in range(NC):
            nc.scalar.activation(
                pT[kt_i][:, c * 512 : (c + 1) * 512],
                pss[:, c * 512 : (c + 1) * 512],
                AF.Exp,
                scale=0.125,
            )
    ps_st.release()

    ps_pv = tc.alloc_tile_pool(name="ps_pv", bufs=2, space="PSUM")
    ps_sum = tc.alloc_tile_pool(name="ps_sum", bufs=2, space="PSUM")
    for sqt in range(NSQ):
        pso = ps_pv.tile([P, D], F32, name="pso", tag="pso")
        psum_s = ps_sum.tile([P, 1], F32, name="psum_s", tag="psum_s")
        for kt_i in range(NSKV):
            lhsT = pT[kt_i][:, sqt * P : (sqt + 1) * P]
            first = kt_i == 0
            last = kt_i == NSKV - 1
            for h in range(2):
                nc.tensor.matmul(
                    pso[:, h * 512 : (h + 1) * 512],
                    lhsT,
                    vt[kt_i][:, h * 512 : (h + 1) * 512],
                    start=first,
                    stop=last,
                )
            nc.tensor.matmul(psum_s[:], lhsT, ones_col[:], start=first, stop=last)
        recip = small.tile([P, 1], F32, name="recip", tag="recip")
        nc.vector.reciprocal(recip[:], psum_s[:])
        ot = outp.tile([P, D], F32, name="ot", tag="ot")
        nc.vector.tensor_scalar_mul(ot[:], pso[:], recip[:])
        nc.sync.dma_start(out.ap()[sqt * P : (sqt + 1) * P, :], ot[:])

    ps_sum.release()
    ps_pv.release()
    small.release()
    outp.release()
    pt_pool.release()
    v_pool.release()
    kt_pool.release()
    qt_pool.release()
    const.release()


_NC_CACHE = {}


def _get_nc(fast):
    if fast not in _NC_CACHE:
        _NC_CACHE[fast] = build(fast=fast)
    return _NC_CACHE[fast]


def kernel(
    hidden_states,
    encoder_hidden_states,
    Wq,
    bq,
    Wk,
    bk,
    Wv,
    bv,
    _trace=False,
    _trace_kwargs=None,
):
    hs = np.ascontiguousarray(np.asarray(hidden_states, np.float32))
    es = np.ascontiguousarray(np.asarray(encoder_hidden_states, np.float32))
    wq_ = np.ascontiguousarray(np.asarray(Wq, np.float32))
    wk_ = np.ascontiguousarray(np.asarray(Wk, np.float32))
    wv_ = np.ascontiguousarray(np.asarray(Wv, np.float32))
    bq_ = np.ascontiguousarray(np.asarray(bq, np.float32))
    bk_ = np.ascontiguousarray(np.asarray(bk, np.float32))
    bv_ = np.ascontiguousarray(np.asarray(bv, np.float32))

    # The S = x (Wq Wk^T) e^T association only absorbs the biases when they
    # are zero; fall back to the general module otherwise.
    fast = not (bq_.any() or bk_.any() or bv_.any())
    nc = _get_nc(fast)
    in_maps = [
        {
            "x": hs[c],
            "e": es[c],
            "wq": wq_,
            "wk": wk_,
            "wv": wv_,
            "bq": bq_,
            "bk": bk_,
            "bv": bv_,
        }
        for c in range(N_CORES)
    ]
    res = run_bass_kernel_spmd(
        nc,
        in_maps,
        list(range(N_CORES)),
        trace=_trace,
        **(_trace_kwargs or {}),
    )
    out = np.stack([res.results[c]["out"] for c in range(N_CORES)], axis=0)
    if _trace:
        return out, res
    return out
